# revision 32
# baseline (speedup 1.0000x reference)
"""CRF loss (forward-algorithm normalizer + tag-sequence score) on 8 trn2 cores.

Math
----
reference loss = sum_b (orig[y[b,0]] + sum_t trans[y[b,t],y[b,t+1]] - normalizer[b])
normalizer[b]  = sum_j alpha_{tau_b}[j, b],  tau_b = batch_sizes[b]-1
alpha_t[j, b]  = x_t[j, b] + logsumexp_k(alpha_{t-1}[k, b] + trans[j, k]),
alpha_0        = x_0 + orig.

Device recursion runs in the exp domain: with ea_t = exp(alpha_t - D_t[b]),
the step is one matmul + one elementwise multiply:

    S_t  = ETT @ ea_{t-1}              # ETT[k, j] = exp(trans[j, k])
    ea_t = exp(x_t) * S_t * r_t        # r_t = RSCALE * rhat every REN steps
    D_t  = D_{t-1} - ln r_t            # rhat recorded in bf16; recorded ==
                                       # applied exactly

Segment parallelism: the serial T-chain is latency-bound (~466ns per
PE->DVE round trip), so T=512 is split into K=16 segments of WS=32 steps
run CONCURRENTLY as extra batch columns (one [128x128]x[128,512] matmul +
one [128,512] multiply per wall step; 31 wall steps instead of 511).
Segment s>0 starts from a uniform guess; by Perron-Frobenius contraction
of positive-matrix products the state direction converges to the true
one within a few steps (validated 1.7e-6 rel err on these inputs in f64).
The per-row scale mismatch at each segment join is exact up to direction
error: ln lambda_s = sum_{u<s} [ln sum_j(seg u end state) - ln RSCALE],
so the device additionally records the final column sums sigma_u; they
enter the same masked ln-sum reduction as the renorm reciprocals, with
host-built weight -1 (events carry +1).

Per-core layout stacks two 32-column batch chains on the 128 partitions
(block-diagonal ETT); free dim = K*32 = 512 segment-columns.  Renorm
events every 8 wall steps (w=8,16,24), prep pipelined 4 steps ahead
(sigma from w-4, reciprocal+record w-3, broadcast matmul w-2, w-scale
emitted before the TT at w-1 so the event step pays no extra latency).

The tag-score side is computed from host-built integer histograms
(count[next, cur] of transition pairs, plus first-tag counts): the device
does sum(count * ptab) where ptab = [trans^T | orig]; parameters are only
ever touched on device.

Sharding: data-parallel over batch, 64 rows per core; per-core partial
sums combined on the host (pure index constants only).
"""

import sys

sys.path.insert(0, "/opt/trn_rl_repo")

import numpy as np
import ml_dtypes

import concourse.bass as bass
import concourse.tile as tile
from concourse import bacc, mybir
from concourse.bass_utils import run_bass_kernel_spmd

# Problem constants (hardcoded per the task contract).
B, T, C = 512, 512, 64
M = 8            # cores
BL = B // M      # 64 batch rows per core
GW = 32          # columns per chain (2 chains stacked on partitions)
K = 16           # time segments run as extra columns
WS = T // K      # 32 wall steps
F = K * GW       # 512 free columns per wall step
REN = 8          # renormalize every REN wall steps
NEVT = WS // REN - 1         # 3 renorm events (w = 8, 16, 24)
RECN = NEVT + 1              # record blocks: events + final sigma
RSCALE = 2.0 ** -40          # renorm down-scale, exact in bf16
LN_RSCALE = float(-40.0 * np.log(2.0))
GVAL = RSCALE / C            # uniform guess entry (exact in bf16)
CH_W = 4         # wall steps of x per DMA chunk
PAD_VAL = -1

f32 = mybir.dt.float32
bf16 = mybir.dt.bfloat16
i16 = mybir.dt.int16
AF = mybir.ActivationFunctionType
ALU = mybir.AluOpType

HF = F // 2      # half-wall width (independent column pipeline)
# packed f32 const columns: trT | orig | ptab | cnt | parm (two half blocks)
PK_TR, PK_OR, PK_PT, PK_CN, PK_PA = 0, 64, 65, 130, 195
PKF_W = 195 + 4 * BL
# packed bf16 const columns: b2 | ones2
PKB_W = 130

_CACHE = {}


def _prefer_combined_act_table():
    """Keep Exp and Ln in one resident activation-function table.

    bacc's table-placement pass assigns each activation the first
    act_func_set containing it; exp and ln default to two different sets,
    so interleaved exp/ln work reloads the 1.3us table each switch.  Both
    live together in the 'natural_log_exp_and_others' set; restricting
    exp/ln membership to that set (names/order unchanged, so set ids stay
    valid) makes the pass emit a single load.
    """
    import concourse.bacc as _bacc

    if getattr(_bacc, "_combined_act_tables", False):
        return
    _orig = _bacc.get_activation_tables

    def _patched(arch):
        tables = _orig(arch)
        keep = "natural_log_exp_and_others"
        if keep in tables:
            both = {AF.Exp, AF.Ln}
            if both <= tables[keep]:
                for name, funcs in tables.items():
                    if name != keep:
                        funcs -= both
        return tables

    _bacc.get_activation_tables = _patched
    _bacc._combined_act_tables = True


def build_program(bench_reps=1):
    key = ("nc", bench_reps)
    if key in _CACHE:
        return _CACHE[key]
    _prefer_combined_act_table()
    nc = bacc.Bacc("TRN2", target_bir_lowering=False, debug=False)

    xT = nc.declare_dram_parameter("xT", [128, WS * F], bf16, isOutput=False)
    pkf = nc.declare_dram_parameter("pkf", [128, PKF_W], f32, isOutput=False)
    pkb = nc.declare_dram_parameter("pkb", [128, PKB_W], bf16, isOutput=False)
    sidx = nc.declare_dram_parameter("sidx", [128, 8], i16, isOutput=False)
    cutm = nc.declare_dram_parameter("cutm", [128, RECN * F], f32, isOutput=False)
    res = nc.declare_dram_parameter("res", [1, 8], f32, isOutput=True)

    NCH = WS // CH_W

    with tile.TileContext(nc) as tc:
        with (
            tc.tile_pool(name="const", bufs=1) as const,
            tc.tile_pool(name="hist", bufs=1) as histp,
            tc.tile_pool(name="xc", bufs=4) as xcpool,
            tc.tile_pool(name="xe", bufs=3) as xepool,
            tc.tile_pool(name="w", bufs=2) as wpool,
            tc.tile_pool(name="post", bufs=1) as post,
            tc.tile_pool(name="psSa", bufs=2, space="PSUM") as psSa,
            tc.tile_pool(name="psSb", bufs=2, space="PSUM") as psSb,
            tc.tile_pool(name="psSig", bufs=1, space="PSUM") as psSig,
            tc.tile_pool(name="psRb", bufs=1, space="PSUM") as psRb,
            tc.tile_pool(name="psFin", bufs=1, space="PSUM") as psFin,
            tc.tile_pool(name="psDum", bufs=1, space="PSUM") as psDum,
        ):
            # ---- wall-0 x first (it gates step 0), then packed constants;
            # trT+orig split out front so ett/eo never wait on the bulky
            # histogram/mask columns ----
            pkf_s = const.tile([128, PKF_W], f32, tag="pkf")
            nc.sync.dma_start(pkf_s[:, 0:PK_PT], pkf[:, 0:PK_PT])
            xc0 = xcpool.tile([128, CH_W * F], bf16, tag="xc")
            nc.sync.dma_start(xc0[:, 0 : 2 * F], xT[:, 0 : 2 * F])
            pkb_s = const.tile([128, PKB_W], bf16, tag="pkb")
            nc.sync.dma_start(pkb_s[:], pkb[:])
            nc.sync.dma_start(xc0[:, 2 * F : CH_W * F], xT[:, 2 * F : CH_W * F])
            nc.sync.dma_start(pkf_s[:, PK_PT:], pkf[:, PK_PT:])

            trT_v = pkf_s[0:C, PK_TR : PK_TR + C]
            orig_v = pkf_s[:, PK_OR : PK_OR + 1]
            ptab_v = pkf_s[0:C, PK_PT : PK_PT + C + 1]
            cnt_v = pkf_s[0:C, PK_CN : PK_CN + C + 1]
            parm_v = pkf_s[:, PK_PA : PK_PA + 4 * BL]
            b2_v = pkb_s[:, 0:128]
            ones2_v = pkb_s[:, 128:130]

            eo = const.tile([128, 1], f32, tag="eo")
            nc.scalar.activation(eo[:], orig_v, AF.Exp)

            # Block-diagonal exp(trans)^T: chain a on partitions a*64..a*64+63.
            ett = const.tile([128, 128], bf16, tag="ett")
            nc.vector.memset(ett[:], 0.0)
            nc.scalar.activation(ett[0:C, 0:C], trT_v, AF.Exp)
            nc.scalar.activation(ett[C:128, C:128], trT_v, AF.Exp)

            ones_col = const.tile([128, 1], f32, tag="ones_col")
            nc.vector.memset(ones_col[:], 1.0)

            # uniform-guess step: q = ETT @ (GVAL * ones)  (per-partition)
            gcol = const.tile([128, 1], bf16, tag="gcol")
            nc.vector.memset(gcol[:], GVAL)
            fin = psFin.tile([128, 10], f32, tag="fin")
            nc.tensor.matmul(fin[:, 8:9], ett[:], gcol[:], start=True, stop=True)
            q = const.tile([128, 1], f32, tag="q")
            nc.vector.tensor_copy(q[:], fin[:, 8:9])

            # records: event r reciprocals at cols [r*F,(r+1)*F), partitions
            # 0:2; final sigma sums at block NEVT.
            rhist = const.tile([2, RECN * F], bf16, tag="rhist")

            itc = const.tile([1, 1], f32, tag="itc")
            nc.vector.memset(itc[:], 0.0)

            # wall history: two PHYSICALLY SEPARATE half tiles so the two
            # half-wall pipelines share no tile-level hazards; half h, step w
            # at columns [w*HF, (w+1)*HF).
            histA = histp.tile([128, WS * HF], bf16, tag="histA")
            histB = histp.tile([128, WS * HF], bf16, tag="histB")
            histH = (histA, histB)

            # ---- record-reduction state + gather priming (before recursion:
            # per-event masked ln-sums run inside recursion slack, and a
            # dummy gather absorbs the gpsimd library-switch drain early) ----
            cutm_s = const.tile([128, RECN * F], f32, tag="cutm")
            sidx_r = const.tile([128, 8], i16, tag="sidx_r")
            # the gpsimd gather wants all inputs written by one engine: DVE stage
            sidx_s = const.tile([128, 8], i16, tag="sidx")
            zidx = const.tile([128, 4], i16, tag="zidx")
            nc.vector.memset(zidx[:], 0)
            dummy = const.tile([128, 2 * BL], bf16, tag="dummy")
            nc.gpsimd.ap_gather(
                dummy[:].rearrange("p (n d) -> p n d", d=2),
                pkb_s[:, 0:128].rearrange("p (n d) -> p n d", d=2),
                zidx[:],
                channels=128, num_elems=64, d=2, num_idxs=BL,
            )

            lnr = post.tile([2, RECN * F], f32, tag="lnr")
            masked = post.tile([2, RECN * F], f32, tag="masked")
            racc = post.tile([2, 4], f32, tag="racc")
            nc.vector.memset(racc[:], 0.0)

            # ---- tag-score from host histograms (independent of recursion) ----
            gmul = post.tile([C, C + 1], f32, tag="gmul")
            gacc = post.tile([C, 1], f32, tag="gacc")
            nc.vector.scalar_tensor_tensor(
                gmul[:], cnt_v, 1.0, ptab_v,
                op0=ALU.mult, op1=ALU.mult, accum_out=gacc[:],
            )
            btot = fin[0:1, 0:1]
            nc.tensor.matmul(
                btot, ones_col[0:C, :], gacc[:], start=True, stop=True
            )

            def emit_recursion():
                chunk_tiles = {}

                def emit_chunk_dma(ci, split=1):
                    xc = xcpool.tile([128, CH_W * F], bf16, tag="xc")
                    w = CH_W * F // split
                    for s in range(split):
                        nc.sync.dma_start(
                            xc[:, s * w : (s + 1) * w],
                            xT[:, ci * CH_W * F + s * w : ci * CH_W * F + (s + 1) * w],
                        )
                    xe = xepool.tile([128, CH_W * F], bf16, tag="xe")
                    chunk_tiles[ci] = (xc, xe)

                def emit_subexp(blk):
                    # exp of x for wall steps [2*blk, 2*blk+2)
                    if 2 * blk >= WS:
                        return
                    ci, sub = divmod(blk, CH_W // 2)
                    xc, xe = chunk_tiles[ci]
                    lo, hi = sub * 2 * F, (sub + 1) * 2 * F
                    nc.scalar.activation(xe[:, lo:hi], xc[:, lo:hi], AF.Exp)

                # chunk 0 was DMA'd up top (wall 0 first); chunks stay
                # 3 ahead of the consuming wall (DMA ~3us/chunk).
                xe0t = xepool.tile([128, CH_W * F], bf16, tag="xe")
                chunk_tiles[0] = (xc0, xe0t)
                emit_chunk_dma(1)
                emit_chunk_dma(2)
                emit_chunk_dma(3)
                # mask/index constants queue behind the x chunks they must
                # not delay; first consumer is the wall-6 event reduction.
                nc.sync.dma_start(cutm_s[:], cutm[:])
                nc.sync.dma_start(sidx_r[:], sidx[:])
                for blk in range(3):
                    emit_subexp(blk)

                # ---- w = 0: seg 0 exact start, segs 1..K-1 guess step ----
                xe0 = chunk_tiles[0][1]
                nc.vector.tensor_scalar_mul(histA[:, 0:GW], xe0[:, 0:GW], eo[:])
                nc.vector.tensor_scalar_mul(histA[:, GW:HF], xe0[:, GW:HF], q[:])
                nc.vector.tensor_scalar_mul(histB[:, 0:HF], xe0[:, HF:F], q[:])

                wt = None
                rr = None
                for w in range(1, WS):
                    ci = w // CH_W
                    if w % CH_W == 0 and ci + 3 < NCH:
                        emit_chunk_dma(ci + 3)
                    if w % 2 == 0:
                        emit_subexp(w // 2 + 2)
                    if w == 4:
                        # stage gather indices here: late enough that the DVE
                        # queue never blocks on the sidx DMA, early enough
                        # that the gpsimd gather-library drain runs inside
                        # the recursion instead of on the tail.
                        nc.vector.tensor_copy(sidx_s[:], sidx_r[:])

                    xecur = chunk_tiles[ci][1]
                    xoff = (w % CH_W) * F
                    if w % REN == 0 and w <= REN * NEVT:
                        win = wt
                    else:
                        win = xecur[:, xoff : xoff + F]

                    ph = w % REN
                    we = w + (REN - ph)
                    prep = we <= REN * NEVT

                    # two half-width pieces in PHYSICALLY separate tiles:
                    # the halves are independent column chains, so TT(half a)
                    # overlaps MM(half b) and the wall period approaches the
                    # DVE-throughput floor.
                    dum = psDum.tile([4, 4], f32, tag="dum")
                    Sa = psSa.tile([128, HF], f32, tag="Sa")
                    Sb = psSb.tile([128, HF], f32, tag="Sb")
                    Sh = (Sa, Sb)
                    for h in range(2):
                        lo = h * HF
                        nc.tensor.matmul(
                            Sh[h][:], ett[:],
                            histH[h][:, (w - 1) * HF : w * HF],
                            start=True, stop=True,
                        )
                        if h == 0 and prep and ph == REN - 1:
                            # w-scale for the event at we=w+1, emitted BEFORE
                            # this wall's TTs: inputs (Rb from w-2, xe) are
                            # ready, so DVE runs it under the matmul in flight.
                            nci, noff = divmod(w + 1, CH_W)
                            wt = wpool.tile([128, F], bf16, tag="wt")
                            nc.vector.tensor_mul(
                                wt[:],
                                chunk_tiles[nci][1][:, noff * F : (noff + 1) * F],
                                Rb[:],
                            )
                        nc.vector.tensor_mul(
                            histH[h][:, w * HF : (w + 1) * HF], Sh[h][:],
                            win[:, lo : lo + HF],
                        )
                        if h == 1:
                            # dependency-free PE keep-alive: without it the
                            # tensor engine idles ~150ns per wall and its
                            # clock never ramps past the mid p-state.
                            nc.tensor.matmul(
                                dum[:], ett[:, 0:4], ett[:, 0:4],
                                start=True, stop=True,
                            )

                    if prep:
                        r = we // REN - 1
                        rcol = r * F
                        if ph == REN - 4:
                            sig = psSig.tile([2, F], f32, tag="sig")
                            for h in range(2):
                                nc.tensor.matmul(
                                    sig[0:2, h * HF : (h + 1) * HF], ones2_v,
                                    histH[h][:, w * HF : (w + 1) * HF],
                                    start=True, stop=True,
                                )
                            rr = wpool.tile([2, F], f32, tag="rr")
                        elif ph == REN - 3:
                            nc.vector.reciprocal_approx_fast(rr[:], sig[:])
                            nc.vector.tensor_copy(
                                rhist[0:2, rcol : rcol + F], rr[:]
                            )
                        elif ph == REN - 2:
                            Rb = psRb.tile([128, F], f32, tag="Rb")
                            lo, hi = r * F, (r + 1) * F
                            nc.scalar.activation(
                                lnr[0:2, lo:hi], rhist[0:2, lo:hi], AF.Ln
                            )
                            nc.vector.scalar_tensor_tensor(
                                masked[0:2, lo:hi], lnr[0:2, lo:hi], 1.0,
                                cutm_s[0:2, lo:hi],
                                op0=ALU.mult, op1=ALU.mult,
                                accum_out=racc[0:2, r : r + 1],
                            )
                            nc.tensor.matmul(
                                Rb[:],
                                b2_v[0:2, :],
                                rhist[0:2, rcol : rcol + F],
                                start=True, stop=True,
                            )

                # ---- record final column sums sigma_u (segment joins) ----
                sigf = psSig.tile([2, F], f32, tag="sig")
                for h in range(2):
                    nc.tensor.matmul(
                        sigf[0:2, h * HF : (h + 1) * HF], ones2_v,
                        histH[h][:, (WS - 1) * HF : WS * HF],
                        start=True, stop=True,
                    )
                nc.vector.tensor_copy(rhist[0:2, NEVT * F : RECN * F], sigf[:])

            def emit_rep():
                nc.vector.tensor_scalar_add(itc[:], itc[:], 1.0)
                emit_recursion()

            if bench_reps == 1:
                emit_rep()
            else:
                with tc.For_i(0, bench_reps, 1):
                    emit_rep()

            # ---- tail: snapshots first (gpsimd path is the long pole),
            # sigma-block reduction overlapped behind them ----
            # ap_gather of column PAIRS (bf16 needs d*size % 4 == 0):
            # out pair-slot b holds hist cols 2*pi, 2*pi+1 for pi = idx//2;
            # the wanted column's parity is folded into the parm mask.
            snapA = post.tile([128, 2 * BL], bf16, tag="snapA")
            snapB = post.tile([128, 2 * BL], bf16, tag="snapB")
            snapln = post.tile([128, 4 * BL], f32, tag="snapln")
            snapsel = post.tile([128, 4 * BL], f32, tag="snapsel")
            saccA = post.tile([128, 1], f32, tag="saccA")
            saccB = post.tile([128, 1], f32, tag="saccB")
            for h, (sn, sc) in enumerate(((snapA, saccA), (snapB, saccB))):
                nc.gpsimd.ap_gather(
                    sn[:].rearrange("p (n d) -> p n d", d=2),
                    histH[h][:].rearrange("p (n d) -> p n d", d=2),
                    sidx_s[:, h * 4 : (h + 1) * 4],
                    channels=128, num_elems=WS * HF // 2, d=2, num_idxs=BL,
                )
                lo, hi = h * 2 * BL, (h + 1) * 2 * BL
                nc.scalar.activation(snapln[:, lo:hi], sn[:], AF.Ln)
                nc.vector.scalar_tensor_tensor(
                    snapsel[:, lo:hi], snapln[:, lo:hi], 1.0,
                    pkf_s[:, PK_PA + lo : PK_PA + hi],
                    op0=ALU.mult, op1=ALU.mult, accum_out=sc[:],
                )

            lo, hi = NEVT * F, RECN * F
            nc.scalar.activation(lnr[0:2, lo:hi], rhist[0:2, lo:hi], AF.Ln)
            nc.vector.scalar_tensor_tensor(
                masked[0:2, lo:hi], lnr[0:2, lo:hi], 1.0, cutm_s[0:2, lo:hi],
                op0=ALU.mult, op1=ALU.mult,
                accum_out=racc[0:2, NEVT : NEVT + 1],
            )

            nA = fin[0:1, 1:2]
            nc.tensor.matmul(nA, ones_col[:], saccA[:], start=True, stop=True)
            nA2 = fin[0:1, 6:7]
            nc.tensor.matmul(nA2, ones_col[:], saccB[:], start=True, stop=True)
            nB = fin[0:1, 2:6]
            nc.tensor.matmul(nB, ones_col[0:2, :], racc[:], start=True, stop=True)

            out_s = post.tile([1, 8], f32, tag="out")
            nc.vector.tensor_copy(out_s[0:1, 0:1], btot)
            nc.vector.tensor_copy(out_s[0:1, 1:2], nA)
            nc.vector.tensor_copy(out_s[0:1, 2:6], nB)
            nc.vector.tensor_copy(out_s[0:1, 6:7], nA2)
            nc.vector.tensor_copy(out_s[0:1, 7:8], itc[:])
            nc.sync.dma_start(res[:], out_s[:])

    nc.compile()
    _CACHE[key] = nc
    return nc


def host_inputs(pad_x, transition_scores, origination_scores, pad_y, batch_sizes):
    """Shard + lay out the full inputs into 8 per-core input maps.

    Host work is limited to data movement and integer index preprocessing;
    every floating-point op on learned parameters / activations runs on
    device.  Returns (in_maps, nev_consts)."""
    pad_x = np.ascontiguousarray(np.asarray(pad_x, dtype=np.float32))
    trans = np.ascontiguousarray(np.asarray(transition_scores, dtype=np.float32))
    origv = np.ascontiguousarray(np.asarray(origination_scores, dtype=np.float32))
    pad_y = np.asarray(pad_y)
    batch_sizes = np.asarray(batch_sizes)

    # x: xT[c][a*64+k, w*F + s*GW + cc] = pad_x[c*64 + a*32 + cc, s*WS + w, k]
    # bf16 on the wire: halves the dominant x stream; the recursion consumes
    # exp(x) in bf16 anyway, so the quantization is in the noise (1.4e-6).
    xr = pad_x.reshape(M, 2, GW, K, WS, C).transpose(0, 1, 5, 4, 3, 2)
    xT = np.ascontiguousarray(xr).reshape(M, 128, WS * F).astype(ml_dtypes.bfloat16)

    y = np.where(pad_y == PAD_VAL, 0, pad_y).astype(np.int64)
    tau = batch_sizes.astype(np.int64) - 1
    sseg = tau // WS            # segment of each row's snapshot
    wall = tau % WS             # wall step within the segment

    pkb = np.zeros((128, PKB_W), np.float32)
    pkb[0, 0:64] = RSCALE          # b2 row 0 -> chain A
    pkb[1, 64:128] = RSCALE        # b2 row 1 -> chain B
    pkb[0:64, 128] = 1.0           # ones2 col 0
    pkb[64:128, 129] = 1.0         # ones2 col 1
    pkb = np.ascontiguousarray(pkb.astype(ml_dtypes.bfloat16))


    in_maps = []
    nevs = []
    for c in range(M):
        yc = y[c * BL : (c + 1) * BL]
        pair = (yc[:, 1:] * C + yc[:, :-1]).reshape(-1)
        cntm = np.bincount(pair, minlength=C * C).astype(np.float32).reshape(C, C)
        ho = np.bincount(yc[:, 0], minlength=C).astype(np.float32).reshape(C, 1)

        pkf = np.zeros((128, PKF_W), np.float32)
        pkf[0:C, PK_TR : PK_TR + C] = trans.T
        pkf[:, PK_OR] = np.concatenate([origv, origv])
        pkf[0:C, PK_PT : PK_PT + C] = trans.T
        pkf[0:C, PK_PT + C] = origv
        pkf[0:C, PK_CN : PK_CN + C] = cntm
        pkf[0:C, PK_CN + C] = ho[:, 0]

        tauc = tau[c * BL : (c + 1) * BL]
        ssegc = sseg[c * BL : (c + 1) * BL]
        wallc = wall[c * BL : (c + 1) * BL]

        # snapshot column of row b within ITS half tile (half = sseg//8):
        # wall*HF + (sseg%8)*GW + (b%GW); rows in the other half point at 0
        # and are masked off via parm.  parity == b%2 (GW even).
        halfc = (ssegc // (K // 2)).astype(np.int64)
        col = wallc * HF + (ssegc % (K // 2)) * GW + (np.arange(BL) % GW)
        idxp = (col // 2).astype(np.int16)
        sidx = np.zeros((128, 8), np.int16)
        parm = np.zeros((128, 4 * BL), np.float32)
        for h in range(2):
            hx = np.where(halfc == h, idxp, 0).astype(np.int16)
            blk = hx.reshape(4, 16).T        # wrapped per 16 partitions
            sidx[:, h * 4 : (h + 1) * 4] = np.tile(blk, (8, 1))
            for b in range(BL):
                if halfc[b] == h:
                    a = b // GW
                    parm[a * 64 : (a + 1) * 64,
                         h * 2 * BL + 2 * b + (b % 2)] = 1.0
        pkf[:, PK_PA : PK_PA + 4 * BL] = parm

        # record weights: event e of segment u counts +1 for row b iff
        # u < sseg_b, or (u == sseg_b and REN*(e+1) <= wall_b); the final
        # sigma of segment u counts -1 iff u < sseg_b.
        cutm = np.zeros((128, RECN * F), np.float32)
        cnt = np.zeros(BL)
        for a in range(2):
            sv = ssegc[a * GW : (a + 1) * GW]   # (GW,)
            wv = wallc[a * GW : (a + 1) * GW]
            for u in range(K):
                base = u * GW
                for e in range(NEVT):
                    w_e = REN * (e + 1)
                    wgt = ((u < sv) | ((u == sv) & (w_e <= wv))).astype(np.float32)
                    cutm[a, e * F + base : e * F + base + GW] = wgt
                    cnt[a * GW : (a + 1) * GW] += wgt
                sw = -((u < sv).astype(np.float32))
                cutm[a, NEVT * F + base : NEVT * F + base + GW] = sw
        # ln(RSCALE) coefficient per row: (+1 events) + sseg  (join -lnRS terms)
        nevs.append(float(cnt.sum() + ssegc.sum()))

        in_maps.append(
            {
                "xT": np.ascontiguousarray(xT[c]),
                "pkf": np.ascontiguousarray(pkf),
                "pkb": pkb,
                "sidx": sidx,
                "cutm": np.ascontiguousarray(cutm),
            }
        )
    return in_maps, nevs


def combine(results, nevs):
    total = 0.0
    for r, nev in zip(results, nevs):
        v = np.asarray(r["res"], dtype=np.float64).reshape(-1)
        # loss_core = score - sum_b normalizer_b
        #           = v0 - (v1 + v6 - C*(sum(v2..v5) + ln(RSCALE)*nev))
        total += (
            v[0] - v[1] - v[6]
            + C * (v[2] + v[3] + v[4] + v[5] + LN_RSCALE * nev)
        )
    return np.asarray(total, dtype=np.float32)


def kernel(pad_x, transition_scores, origination_scores, pad_y, batch_sizes):
    nc = build_program()
    in_maps, nevs = host_inputs(
        pad_x, transition_scores, origination_scores, pad_y, batch_sizes
    )
    out = run_bass_kernel_spmd(nc, in_maps, core_ids=list(range(M)))
    return combine(out.results, nevs)
